# revision 1
# baseline (speedup 1.0000x reference)
"""Trainium2 Bass kernel for nn_Decoder_38757784879455 (GNN message passing decoder).

Strategy (8 NeuronCores, data-parallel over the 32 scenes => 4 scenes/core):
  * Host folds the tiny input MLPs into the first hidden layer:
      h1_pre[:, (i,j)] = QY[:, j] - q[:, i]      (per scene)
    where q = A^T @ [px;py;vx;vy], A = [Ws@Wm1_pe; Wv@Wm1_ve] (host, f64),
    QY = q + Wm1_hid^T @ hid^T + beff.  This removes the big first matmul
    entirely (it becomes a broadcasted subtract).
  * Visibility mask via closed form: cos(deg)=yr/r, sin(deg)=-xr/r (no arctan).
  * Masked max/min pooling via +-BIG rank-1 PSUM bias matmuls (exact: valid
    lanes untouched, invalid lanes pushed out of the reduction range), then
    relu/bias applied to the tiny reduced (256 x 64) tensors only.
  * Second MLP layer (the only remaining big matmul) runs on the PE.
"""

import math

import numpy as np

import concourse.bass as bass
import concourse.mybir as mybir
import concourse.tile as tile
from concourse import bacc
from concourse.bass_utils import run_bass_kernel_spmd

# problem constants
E = 64
H = 128
D = 256
MLP = 512
B_SEQ = 32
P = 64
N = B_SEQ * P
NCORES = 8
S = B_SEQ // NCORES          # scenes per core
NP_CORE = S * P              # pedestrians per core

BIG = 1.0e4
DEG_VISION = 120.0
_half = DEG_VISION / 2.0
BCONE = math.sin(math.radians(_half)) * (2.0 / math.cos(math.radians(_half)))

FP = mybir.dt.float32
ALU = mybir.AluOpType
ACTF = mybir.ActivationFunctionType

# matmul input dtype for the heavy matmuls (float32 = exact, 4 cyc/row;
# float32r = 1 cyc/row at N>=256 with reduced multiply precision)
import os
F32R = os.environ.get("F32R", "1") == "1"
FR = mybir.dt.float32r
H1DT = FR if F32R else FP

# packed-input column layout (all fp32, one (128, ACOLS) tensor)
C_HIDT = 0
C_WM2 = C_HIDT + NP_CORE          # 4*D cols
C_WP = C_WM2 + 4 * D
C_WM1H = C_WP + 4 * D
C_A4 = C_WM1H + MLP
C_GEO = C_A4 + MLP
C_GEOT = C_GEO + NP_CORE
C_NOTI = C_GEOT + 8 * S
C_BEFF = C_NOTI + P
C_BM2 = C_BEFF + 4
C_BP = C_BM2 + 2
C_CONSTS = C_BP + D
ACOLS = ((C_CONSTS + 384 + 127) // 128) * 128


def _mm_cast(ap):
    return ap


def build_program(reps=1):
    """Builds the per-core Bass program (same program on all 8 cores).

    reps>1 repeats the whole computation on-device (for differential
    wall-clock timing of the device portion)."""
    nc = bacc.Bacc(None, target_bir_lowering=False, debug=False)

    # ---- DRAM I/O ----
    # All inputs packed into ONE (128, ACOLS) tensor -> ONE striped DMA
    # (1 DMA = 1 semaphore: keeps every matmul at <=1 sync wait, and a single
    #  >=1MiB DMA stripes across all 16 SDMA engines at ~340 GB/s).
    allin = nc.dram_tensor("allin", [128, ACOLS], FP, kind="ExternalInput").ap()
    outp = nc.dram_tensor("outp", [NP_CORE, D], FP, kind="ExternalOutput").ap()

    with tile.TileContext(nc) as tc:
        with (
            tc.tile_pool(name="singles", bufs=1) as singles,
            tc.tile_pool(name="geom", bufs=2) as geom,
            tc.tile_pool(name="rows", bufs=2) as rows,
            tc.tile_pool(name="qy", bufs=2) as qyp,
            tc.tile_pool(name="h1", bufs=6) as h1p,
            tc.tile_pool(name="small", bufs=4) as small,
            tc.tile_pool(name="outs", bufs=2) as outsp,
            tc.tile_pool(name="psq", bufs=1, space="PSUM") as psq,
            tc.tile_pool(name="psh2", bufs=3, space="PSUM") as psh2,
            tc.tile_pool(name="pssm", bufs=1, space="PSUM") as pssm,
            tc.tile_pool(name="dram", bufs=2, space="DRAM") as dramp,
        ):
            # ---- load everything in one DMA ----
            allin_sb = singles.tile([128, ACOLS], FP)
            nc.sync.dma_start(out=allin_sb[:], in_=allin)
            hidT_sb = allin_sb[0:H, C_HIDT : C_HIDT + NP_CORE]
            wm2_sb = allin_sb[:, C_WM2 : C_WM2 + 4 * D]
            wp_sb = allin_sb[:, C_WP : C_WP + 4 * D]
            wm1h_sb = allin_sb[0:H, C_WM1H : C_WM1H + MLP]
            a4_sb = allin_sb[0:4, C_A4 : C_A4 + MLP]
            geo_sb = allin_sb[0:8, C_GEO : C_GEO + NP_CORE]
            geoT_sb = allin_sb[0:P, C_GEOT : C_GEOT + 8 * S]
            noti_sb = allin_sb[0:P, C_NOTI : C_NOTI + P]
            beff_sb = allin_sb[:, C_BEFF : C_BEFF + 4]
            bm2_sb = allin_sb[:, C_BM2 : C_BM2 + 2]
            bprow_sb = allin_sb[0:1, C_BP : C_BP + D]
            consts_sb = allin_sb[0:1, C_CONSTS : C_CONSTS + 384]

            constr_sb = singles.tile([1, 384], H1DT)
            nc.vector.tensor_copy(out=constr_sb[:], in_=allin_sb[0:1, C_CONSTS : C_CONSTS + 384])
            if F32R:
                wm2r_sb = singles.tile([128, 4 * D], FR)
                nc.vector.tensor_copy(out=wm2r_sb[:], in_=wm2_sb)
                wpr_sb = singles.tile([128, 4 * D], FR)
                nc.vector.tensor_copy(out=wpr_sb[:], in_=wp_sb)
            else:
                wm2r_sb, wpr_sb = wm2_sb, wp_sb

            negbig_row = constr_sb[0:1, 0:128]
            pos2big_row = constr_sb[0:1, 128:256]
            ones_row = consts_sb[0:1, 256:384]

            for rep in range(reps):
             for s in range(S):
                s8 = s * 8
                c0 = s * P            # column base for this scene

                # ======== stage A: geometry / visibility mask ========
                col = lambda r: geoT_sb[:, s8 + r : s8 + r + 1]
                xr = geom.tile([P, 1], FP)
                nc.vector.tensor_tensor(out=xr[:], in0=col(0), in1=col(4), op=ALU.subtract)
                yr = geom.tile([P, 1], FP)
                nc.vector.tensor_tensor(out=yr[:], in0=col(1), in1=col(5), op=ALU.subtract)
                r2 = geom.tile([P, 1], FP)
                # r2 = xr*xr + yr*yr  (two tiny ops)
                nc.vector.tensor_tensor(out=r2[:], in0=xr[:], in1=xr[:], op=ALU.mult)
                yr2 = geom.tile([P, 1], FP)
                nc.vector.tensor_tensor(out=yr2[:], in0=yr[:], in1=yr[:], op=ALU.mult)
                nc.vector.tensor_tensor(out=r2[:], in0=r2[:], in1=yr2[:], op=ALU.add)
                # clamp away exact zero then rsqrt + one Newton step
                nc.vector.tensor_scalar(out=r2[:], in0=r2[:], scalar1=1e-30, scalar2=None, op0=ALU.max)
                rr = geom.tile([P, 1], FP)
                nc.scalar.activation(out=rr[:], in_=r2[:], func=ACTF.Sqrt)
                rinv = geom.tile([P, 1], FP)
                nc.vector.reciprocal(out=rinv[:], in_=rr[:])
                cd = geom.tile([P, 1], FP)
                nc.vector.tensor_tensor(out=cd[:], in0=yr[:], in1=rinv[:], op=ALU.mult)
                msd = geom.tile([P, 1], FP)   # msd = -sd = xr*rinv
                nc.vector.tensor_tensor(out=msd[:], in0=xr[:], in1=rinv[:], op=ALU.mult)
                sd = geom.tile([P, 1], FP)
                nc.vector.tensor_scalar(out=sd[:], in0=msd[:], scalar1=-1.0, scalar2=None, op0=ALU.mult)

                # replicated position rows: PJ[p, r, j] = geo[r, c0+j], r in {0,1}
                pj = geom.tile([P, 2, P], FP, bufs=4)
                src = bass.AP(
                    tensor=allin.tensor,
                    offset=allin.offset + C_GEO + c0,
                    ap=[[0, P], [ACOLS, 2], [1, P]],
                )
                nc.gpsimd.dma_start(out=pj[:], in_=src)

                dx = geom.tile([P, P], FP)
                nc.vector.tensor_scalar(out=dx[:], in0=pj[:, 0, :], scalar1=col(0), scalar2=None, op0=ALU.subtract)
                dy = geom.tile([P, P], FP)
                nc.vector.tensor_scalar(out=dy[:], in0=pj[:, 1, :], scalar1=col(1), scalar2=None, op0=ALU.subtract)
                # x_t = cd*dx - sd*dy ; y_t = sd*dx + cd*dy
                t1 = geom.tile([P, P], FP)
                nc.vector.tensor_scalar(out=t1[:], in0=dx[:], scalar1=cd[:], scalar2=None, op0=ALU.mult)
                x_t = geom.tile([P, P], FP)
                nc.vector.scalar_tensor_tensor(out=x_t[:], in0=dy[:], scalar=msd[:], in1=t1[:], op0=ALU.mult, op1=ALU.add)
                t2 = geom.tile([P, P], FP)
                nc.vector.tensor_scalar(out=t2[:], in0=dy[:], scalar1=cd[:], scalar2=None, op0=ALU.mult)
                y_t = geom.tile([P, P], FP)
                nc.vector.scalar_tensor_tensor(out=y_t[:], in0=dx[:], scalar=sd[:], in1=t2[:], op0=ALU.mult, op1=ALU.add)

                ypos = geom.tile([P, P], FP)
                nc.vector.tensor_scalar(out=ypos[:], in0=y_t[:], scalar1=0.0, scalar2=None, op0=ALU.is_ge)
                w_t = geom.tile([P, P], FP)
                nc.vector.tensor_scalar(out=w_t[:], in0=ypos[:], scalar1=-0.75, scalar2=1.0, op0=ALU.mult, op1=ALU.add)
                x2 = geom.tile([P, P], FP)
                nc.vector.tensor_tensor(out=x2[:], in0=x_t[:], in1=x_t[:], op=ALU.mult)
                y2 = geom.tile([P, P], FP)
                nc.vector.tensor_tensor(out=y2[:], in0=y_t[:], in1=y_t[:], op=ALU.mult)
                y2w = geom.tile([P, P], FP)
                nc.vector.tensor_tensor(out=y2w[:], in0=y2[:], in1=w_t[:], op=ALU.mult)
                res = geom.tile([P, P], FP)
                nc.vector.tensor_tensor(out=res[:], in0=x2[:], in1=y2w[:], op=ALU.add)
                egg = geom.tile([P, P], FP)
                nc.vector.tensor_scalar(out=egg[:], in0=res[:], scalar1=1.0, scalar2=None, op0=ALU.is_le)

                tx2 = geom.tile([P, P], FP)
                nc.vector.tensor_scalar(out=tx2[:], in0=x_t[:], scalar1=2.0, scalar2=None, op0=ALU.mult)
                c1 = geom.tile([P, P], FP)
                nc.vector.scalar_tensor_tensor(out=c1[:], in0=y_t[:], scalar=BCONE, in1=tx2[:], op0=ALU.mult, op1=ALU.add)
                c2 = geom.tile([P, P], FP)
                nc.vector.scalar_tensor_tensor(out=c2[:], in0=y_t[:], scalar=-BCONE, in1=tx2[:], op0=ALU.mult, op1=ALU.add)
                g1 = geom.tile([P, P], FP)
                nc.vector.tensor_scalar(out=g1[:], in0=c1[:], scalar1=0.0, scalar2=None, op0=ALU.is_gt)
                l2 = geom.tile([P, P], FP)
                nc.vector.tensor_scalar(out=l2[:], in0=c2[:], scalar1=0.0, scalar2=None, op0=ALU.is_lt)
                cone = geom.tile([P, P], FP)
                nc.vector.tensor_tensor(out=cone[:], in0=g1[:], in1=l2[:], op=ALU.mult)
                z1 = geom.tile([P, P], FP)
                nc.vector.tensor_scalar(out=z1[:], in0=c1[:], scalar1=0.0, scalar2=None, op0=ALU.is_equal)
                nc.vector.tensor_tensor(out=cone[:], in0=cone[:], in1=z1[:], op=ALU.max)
                nc.vector.tensor_scalar(out=z1[:], in0=c2[:], scalar1=0.0, scalar2=None, op0=ALU.is_equal)
                nc.vector.tensor_tensor(out=cone[:], in0=cone[:], in1=z1[:], op=ALU.max)

                mask = geom.tile([P, P], FP)
                nc.vector.tensor_tensor(out=mask[:], in0=egg[:], in1=ypos[:], op=ALU.mult)
                nc.vector.tensor_tensor(out=mask[:], in0=mask[:], in1=cone[:], op=ALU.mult)
                nc.vector.tensor_tensor(out=mask[:], in0=mask[:], in1=noti_sb, op=ALU.mult)

                notmask = geom.tile([P, P], H1DT)
                nc.vector.tensor_scalar(out=notmask[:], in0=mask[:], scalar1=-1.0, scalar2=1.0, op0=ALU.mult, op1=ALU.add)

                # flatten notmask (64,64) -> (1, 4096) and has (64,1) -> (1,64)
                # via DRAM round-trip (cross-partition reshape)
                nm_dr = dramp.tile([P, P], H1DT, name="nmdr", bufs=4)
                nc.gpsimd.dma_start(out=nm_dr[:], in_=notmask[:])
                nm_row = rows.tile([1, P * P], H1DT, name="nmrow", bufs=4)
                nc.gpsimd.dma_start(
                    out=nm_row[:],
                    in_=nm_dr[:].rearrange("p j -> (p j)").rearrange("(a c) -> a c", a=1),
                )
                # ======== stage B: QY / q ========
                q_sb = []
                qy_sb = []
                for mt in range(4):
                    ps_q = psq.tile([128, P], FP, tag="psq", name="psq")
                    nc.tensor.matmul(
                        ps_q[:],
                        a4_sb[:, mt * 128 : (mt + 1) * 128],
                        geo_sb[0:4, c0 : c0 + P],
                        start=True, stop=False,
                    )
                    qt = qyp.tile([128, P], FP, name=f"qt{mt}")
                    nc.scalar.activation(out=qt[:], in_=ps_q[:], func=ACTF.Copy)
                    nc.tensor.matmul(
                        ps_q[:],
                        wm1h_sb[:, mt * 128 : (mt + 1) * 128],
                        hidT_sb[:, c0 : c0 + P],
                        start=False, stop=True,
                    )
                    qyt = qyp.tile([128, P], FP, name=f"qyt{mt}")
                    nc.scalar.activation(
                        out=qyt[:], in_=ps_q[:], func=ACTF.Identity,
                        bias=beff_sb[:, mt : mt + 1],
                    )
                    q_sb.append(qt)
                    qy_sb.append(qyt)

                gmax = [small.tile([128, P], FP, name=f"gmax{m}", bufs=2) for m in range(2)]
                umin = [small.tile([128, P], FP, name=f"umin{m}", bufs=2) for m in range(2)]

                # ======== per column block: h1, mm2, masked reduces ========
                # 16 pedestrians (i) per block -> (128, 1024) tiles, 2 PSUM banks
                for blk in range(4):
                    i0 = blk * 16
                    h1f = []
                    for mt in range(4):
                        h1pre = h1p.tile([128, 16, P], FP, tag="h1pre", bufs=6, name="h1pre")
                        qy_b = qy_sb[mt][:].rearrange("p (a j) -> p a j", a=1).to_broadcast([128, 16, P])
                        q_b = (
                            q_sb[mt][:, i0 : i0 + 16]
                            .rearrange("p (a j) -> p a j", j=1)
                            .to_broadcast([128, 16, P])
                        )
                        eng = nc.gpsimd if mt == 3 else nc.vector
                        eng.tensor_tensor(out=h1pre[:], in0=qy_b, in1=q_b, op=ALU.subtract)
                        h1t = h1p.tile([128, 16 * P], H1DT, tag="h1f", bufs=8, name="h1f")
                        nc.scalar.activation(
                            out=h1t[:], in_=h1pre[:].rearrange("p a j -> p (a j)"),
                            func=ACTF.Relu,
                        )
                        h1f.append(h1t)

                    nm_slice = [nm_row[0:1, (i0 + h * 8) * P : (i0 + h * 8 + 8) * P] for h in range(2)]
                    for m2 in range(2):
                        ps_h2 = psh2.tile([128, 16 * P], FP, tag="psh2", name="psh2")
                        for kc in range(4):
                            for h in range(2):
                                nc.tensor.matmul(
                                    ps_h2[:, h * 512 : (h + 1) * 512],
                                    wm2r_sb[:, kc * D + m2 * 128 : kc * D + (m2 + 1) * 128] if F32R
                                    else wm2_sb[:, kc * D + m2 * 128 : kc * D + (m2 + 1) * 128],
                                    h1f[kc][:, h * 512 : (h + 1) * 512],
                                    start=(kc == 0), stop=False,
                                )
                        # max-path bias: psum += -BIG * (1-mask)
                        for h in range(2):
                            nc.tensor.matmul(ps_h2[:, h * 512 : (h + 1) * 512], negbig_row, nm_slice[h], start=False, stop=False)
                        nc.vector.tensor_reduce(
                            out=gmax[m2][:, i0 : i0 + 16],
                            in_=ps_h2[:].rearrange("p (a j) -> p a j", j=P),
                            axis=mybir.AxisListType.X, op=ALU.max,
                        )
                        # min-path bias: psum += +2BIG * (1-mask)  (net +BIG)
                        for h in range(2):
                            nc.tensor.matmul(ps_h2[:, h * 512 : (h + 1) * 512], pos2big_row, nm_slice[h], start=False, stop=(h == 1))
                        nc.vector.tensor_reduce(
                            out=umin[m2][:, i0 : i0 + 16],
                            in_=ps_h2[:].rearrange("p (a j) -> p a j", j=P),
                            axis=mybir.AxisListType.X, op=ALU.min,
                        )

                # ======== scene tail: pooled -> output ========
                pooled = []
                for m2 in range(2):
                    mx = small.tile([128, P], H1DT, name=f"mx{m2}")
                    nc.scalar.activation(
                        out=mx[:], in_=gmax[m2][:], func=ACTF.Relu,
                        bias=bm2_sb[:, m2 : m2 + 1],
                    )
                    pooled.append(mx)
                for m2 in range(2):
                    mnr = small.tile([128, P], FP, name=f"mnr{m2}")
                    nc.scalar.activation(
                        out=mnr[:], in_=umin[m2][:], func=ACTF.Relu,
                        bias=bm2_sb[:, m2 : m2 + 1],
                    )
                    sel = small.tile([128, P], FP, name=f"sel{m2}")
                    nc.vector.tensor_scalar(out=sel[:], in0=mnr[:], scalar1=5000.0, scalar2=None, op0=ALU.is_lt)
                    mn = small.tile([128, P], H1DT, name=f"mn{m2}")
                    nc.vector.tensor_tensor(out=mn[:], in0=mnr[:], in1=sel[:], op=ALU.mult)
                    pooled.append(mn)

                ps_o = pssm.tile([P, D], FP, tag="pso", name="pso")
                for kc in range(4):
                    nc.tensor.matmul(
                        ps_o[:],
                        pooled[kc][:],
                        wpr_sb[:, kc * D : (kc + 1) * D] if F32R else wp_sb[:, kc * D : (kc + 1) * D],
                        start=(kc == 0), stop=False,
                    )
                nc.tensor.matmul(ps_o[:], ones_row[0:1, 0:P], bprow_sb, start=False, stop=True)
                out_sb = outsp.tile([P, D], FP, name="outsb")
                nc.scalar.activation(out=out_sb[:], in_=ps_o[:], func=ACTF.Relu)
                nc.sync.dma_start(out=outp[c0 : c0 + P, :], in_=out_sb[:])

    nc.finalize()
    return nc


def _host_prep(h_states, seq_start_end, end_pos, end_velocity, before_end_pos,
               W_s, b_s, W_v, b_v, Wm1, bm1, Wm2, bm2, Wp, bp):
    """Fold weights (f64) and pack per-core input maps."""
    f64 = np.float64
    A = np.concatenate(
        [W_s.astype(f64) @ Wm1[:E].astype(f64),
         W_v.astype(f64) @ Wm1[E : 2 * E].astype(f64)], axis=0
    ).astype(np.float32)                                      # (4, 512)
    beff = (bm1.astype(f64) + b_s.astype(f64) @ Wm1[:E].astype(f64)
            + b_v.astype(f64) @ Wm1[E : 2 * E].astype(f64)).astype(np.float32)
    Wm1h = np.ascontiguousarray(Wm1[2 * E :])                 # (128, 512)

    # Wm2 (512, 256) -> (128, 4*256): [:, kc*256 + m] = Wm2[kc*128 + p, m]
    wm2p = np.ascontiguousarray(
        Wm2.reshape(4, 128, D).transpose(1, 0, 2).reshape(128, 4 * D)
    )
    wpp = np.ascontiguousarray(
        Wp.reshape(4, 128, D).transpose(1, 0, 2).reshape(128, 4 * D)
    )
    beff_pack = np.ascontiguousarray(beff.reshape(4, 128).T)  # (128, 4)
    bm2_pack = np.ascontiguousarray(bm2.reshape(2, 128).T)    # (128, 2)
    bp_row = np.ascontiguousarray(bp.reshape(1, D))

    consts = np.zeros((1, 384), np.float32)
    consts[0, 0:128] = -BIG
    consts[0, 128:256] = 2.0 * BIG
    consts[0, 256:384] = 1.0
    noti = (1.0 - np.eye(P, dtype=np.float32))

    pos = end_pos.reshape(B_SEQ, P, 2)
    vel = end_velocity.reshape(B_SEQ, P, 2)
    bef = before_end_pos.reshape(B_SEQ, P, 2)
    hid = h_states.reshape(B_SEQ, P, H)

    in_maps = []
    for c in range(NCORES):
        sl = slice(c * S, (c + 1) * S)
        p_, v_, b_ = pos[sl], vel[sl], bef[sl]          # (S, P, 2)
        geo = np.zeros((8, NP_CORE), np.float32)
        geo[0] = p_[..., 0].reshape(-1)
        geo[1] = p_[..., 1].reshape(-1)
        geo[2] = v_[..., 0].reshape(-1)
        geo[3] = v_[..., 1].reshape(-1)
        geo[4] = b_[..., 0].reshape(-1)
        geo[5] = b_[..., 1].reshape(-1)
        geoT = np.ascontiguousarray(
            geo.reshape(8, S, P).transpose(2, 1, 0).reshape(P, S * 8)
        )
        hidT = np.ascontiguousarray(hid[sl].reshape(NP_CORE, H).T)  # (128, S*64)

        allin = np.zeros((128, ACOLS), np.float32)
        allin[0:H, C_HIDT : C_HIDT + NP_CORE] = hidT
        allin[:, C_WM2 : C_WM2 + 4 * D] = wm2p
        allin[:, C_WP : C_WP + 4 * D] = wpp
        allin[0:H, C_WM1H : C_WM1H + MLP] = Wm1h
        allin[0:4, C_A4 : C_A4 + MLP] = A
        allin[0:8, C_GEO : C_GEO + NP_CORE] = geo
        allin[0:P, C_GEOT : C_GEOT + 8 * S] = geoT
        allin[0:P, C_NOTI : C_NOTI + P] = noti
        allin[:, C_BEFF : C_BEFF + 4] = beff_pack
        allin[:, C_BM2 : C_BM2 + 2] = bm2_pack
        allin[0:1, C_BP : C_BP + D] = bp_row
        allin[0:1, C_CONSTS : C_CONSTS + 384] = consts
        in_maps.append({"allin": allin})
    return in_maps


_CACHED_NC = None


def kernel(**inputs):
    global _CACHED_NC
    inputs = {k: np.asarray(v) for k, v in inputs.items()}
    in_maps = _host_prep(**inputs)
    if _CACHED_NC is None:
        _CACHED_NC = build_program()
    res = run_bass_kernel_spmd(_CACHED_NC, in_maps, core_ids=list(range(NCORES)))
    out = np.concatenate([r["outp"] for r in res.results], axis=0)
    return out.astype(np.float32)


if __name__ == "__main__":
    np.random.seed(0)
    fake = {
        "h_states": np.random.randn(1, N, H).astype(np.float32),
        "seq_start_end": np.stack(
            [np.arange(B_SEQ, dtype=np.int32) * P,
             (np.arange(B_SEQ, dtype=np.int32) + 1) * P], axis=1),
        "end_pos": (np.random.rand(N, 2) * 8).astype(np.float32),
        "end_velocity": (0.5 * np.random.randn(N, 2)).astype(np.float32),
        "before_end_pos": np.random.randn(N, 2).astype(np.float32),
        "W_s": np.random.randn(2, E).astype(np.float32) * 0.5,
        "b_s": np.random.randn(E).astype(np.float32) * 0.5,
        "W_v": np.random.randn(2, E).astype(np.float32) * 0.5,
        "b_v": np.random.randn(E).astype(np.float32) * 0.5,
        "Wm1": (np.random.randn(2 * E + H, MLP) / 16).astype(np.float32),
        "bm1": (np.random.randn(MLP) / 16).astype(np.float32),
        "Wm2": (np.random.randn(MLP, D) / 22).astype(np.float32),
        "bm2": (np.random.randn(D) / 22).astype(np.float32),
        "Wp": (np.random.randn(2 * D, D) / 22).astype(np.float32),
        "bp": (np.random.randn(D) / 22).astype(np.float32),
    }
    out = kernel(**fake)
    print("kernel ran, out", out.shape, out.dtype, float(np.abs(out).max()))

